# revision 3
# baseline (speedup 1.0000x reference)
"""Trainium2 Bass kernel for nn_MeanAggregator (segment mean + time features).

Computation (see reference):
  out[e, p, 0:256]   = mean of 10 gathered ent_embeds rows of segment 5e+p   (p < 5)
  out[e, p, 256:288] = cos(t * t_w + t_b), t = time_vals[5e+p]               (p < 5)
  out[e, p, 0:256]   = 0                                                      (p >= 5)
  out[e, p, 256:288] = cos(1e6 * t_w + t_b)                                   (p >= 5)

Sharding: data-parallel over examples; core c owns examples [2500c, 2500(c+1)).

Device work per core: 125k-row gather via indirect DMA (one row per
partition, example-native order so no reorder is needed), spread across the
4 SWDGE queues so descriptor generation on the Q7 pairs overlaps; the table
is pre-converted to bf16 on the host to halve gather bytes; a pairwise add
tree (bf16 -> f32) forms the 10-row segment sums; ScalarE scales into the
output tile; time features come from a host-side 300-entry cos LUT.
"""

import math
import os
import sys

import numpy as np

sys.path.insert(0, "/opt/trn_rl_repo")

from contextlib import ExitStack

import ml_dtypes

import concourse.bass as bass
import concourse.tile as tile
from concourse import bacc, mybir
from concourse._compat import with_exitstack
from concourse.bass_utils import run_bass_kernel_spmd

# Problem constants (hardcoded; kernel.py must be self-contained).
N_CORES = 8
NUM_ENTITIES = 200000
H = 256
T = 32
SEQ_LEN = 10
N_EXAMPLES = 20000
SEGS_PER_EX = 5
NODES_PER_SEG = 10
EX_PER_CORE = N_EXAMPLES // N_CORES  # 2500
P = 128
NBLK = (EX_PER_CORE + P - 1) // P  # 20
PAD_TIME = 1000000.0

N_QUEUES = 4
_QNAMES = ["qPoolDynamic", "qPoolDynamic1", "qPoolDynamic2", "qPoolDynamic3"]

_CACHE = {}


@with_exitstack
def _emit(ctx: ExitStack, tc, table, idx, tf, padfull, out):
    nc = tc.nc
    f32 = mybir.dt.float32
    bf16 = mybir.dt.bfloat16

    const_pool = ctx.enter_context(tc.tile_pool(name="const", bufs=1))
    g_pool = ctx.enter_context(tc.tile_pool(name="g", bufs=64))
    io_pool = ctx.enter_context(tc.tile_pool(name="io", bufs=3))
    out_pool = ctx.enter_context(tc.tile_pool(name="outp", bufs=3))
    acc_pool = ctx.enter_context(tc.tile_pool(name="acc", bufs=16))

    pad_t = const_pool.tile([P, SEGS_PER_EX, H + T], f32)
    nc.sync.dma_start(out=pad_t[:], in_=padfull)

    qi = 0
    for b in range(NBLK):
        npar = min(P, EX_PER_CORE - b * P)
        idx_t = io_pool.tile([P, SEGS_PER_EX * NODES_PER_SEG], mybir.dt.int32)
        nc.sync.dma_start(out=idx_t[:npar], in_=idx[b, :npar])
        out_t = out_pool.tile([P, SEGS_PER_EX, H + T], f32)
        nc.sync.dma_start(out=out_t[:npar, :, H : H + T], in_=tf[b, :npar])

        gsj = []
        for j in range(SEGS_PER_EX):
            gs = []
            for k in range(NODES_PER_SEG):
                c = j * NODES_PER_SEG + k
                g = g_pool.tile([P, H], bf16)
                # HW indirect DMA only honors [P, 1] offset APs (one index
                # per partition); multi-index offsets gather garbage.
                inst = nc.gpsimd.indirect_dma_start(
                    out=g[:npar],
                    out_offset=None,
                    in_=table,
                    in_offset=bass.IndirectOffsetOnAxis(
                        ap=idx_t[:npar, c : c + 1], axis=0
                    ),
                )
                inst.ins.queue = _QNAMES[qi % N_QUEUES]
                qi += 1
                gs.append(g)
            gsj.append(gs)
        for j in range(SEGS_PER_EX):
            gs = gsj[j]
            # pairwise tree: 5 bf16+bf16->f32 adds, then 4 f32 adds
            lvl = []
            for k in range(0, NODES_PER_SEG, 2):
                s = acc_pool.tile([P, H], f32)
                nc.vector.tensor_tensor(
                    out=s[:npar], in0=gs[k][:npar], in1=gs[k + 1][:npar],
                    op=mybir.AluOpType.add,
                )
                lvl.append(s)
            a = acc_pool.tile([P, H], f32)
            nc.vector.tensor_tensor(
                out=a[:npar], in0=lvl[0][:npar], in1=lvl[1][:npar],
                op=mybir.AluOpType.add,
            )
            bsum = acc_pool.tile([P, H], f32)
            nc.vector.tensor_tensor(
                out=bsum[:npar], in0=lvl[2][:npar], in1=lvl[3][:npar],
                op=mybir.AluOpType.add,
            )
            nc.vector.tensor_tensor(
                out=a[:npar], in0=a[:npar], in1=bsum[:npar],
                op=mybir.AluOpType.add,
            )
            nc.vector.tensor_tensor(
                out=a[:npar], in0=a[:npar], in1=lvl[4][:npar],
                op=mybir.AluOpType.add,
            )
            nc.scalar.mul(out_t[:npar, j, 0:H], a[:npar], 1.0 / NODES_PER_SEG)

        rows = slice(b * P, b * P + npar)
        nc.sync.dma_start(out=out[rows, 0:SEGS_PER_EX, :], in_=out_t[:npar])
        nc.sync.dma_start(out=out[rows, SEGS_PER_EX:SEQ_LEN, :], in_=pad_t[:npar])


def _build_nc():
    nc = bacc.Bacc(
        "TRN2",
        target_bir_lowering=False,
        debug=False,
        enable_asserts=False,
        num_devices=N_CORES,
        num_swdge_queues=N_QUEUES,
    )
    f32 = mybir.dt.float32
    bf16 = mybir.dt.bfloat16
    table = nc.dram_tensor("table", [NUM_ENTITIES, H], bf16, kind="ExternalInput").ap()
    idx = nc.dram_tensor(
        "idx", [NBLK, P, SEGS_PER_EX * NODES_PER_SEG], mybir.dt.int32,
        kind="ExternalInput",
    ).ap()
    tf = nc.dram_tensor(
        "tf", [NBLK, P, SEGS_PER_EX, T], f32, kind="ExternalInput"
    ).ap()
    padfull = nc.dram_tensor(
        "padf", [P, SEGS_PER_EX, H + T], f32, kind="ExternalInput"
    ).ap()
    out = nc.dram_tensor(
        "out", [EX_PER_CORE, SEQ_LEN, H + T], f32, kind="ExternalOutput"
    ).ap()
    with tile.TileContext(nc) as tc:
        _emit(tc, table, idx, tf, padfull, out)
    nc.compile()
    return nc


def kernel(
    ent_embeds, t_w, t_b, flat_s, node_seg_ids, seg_example, seg_pos, time_vals
):
    ent_embeds = np.ascontiguousarray(ent_embeds, dtype=np.float32)
    t_w = np.asarray(t_w, dtype=np.float32)
    t_b = np.asarray(t_b, dtype=np.float32)
    flat_s = np.asarray(flat_s, dtype=np.int32)
    time_vals = np.asarray(time_vals, dtype=np.int32)

    if "nc" not in _CACHE:
        _CACHE["nc"] = _build_nc()
    nc = _CACHE["nc"]

    table_bf16 = np.ascontiguousarray(ent_embeds.astype(ml_dtypes.bfloat16))

    # Host-side prep. Time features take only 300 distinct integer t values:
    # precompute the 300x32 cos LUT (like an activation table) and expand.
    tmax = int(time_vals.max()) + 1
    lut = np.cos(
        np.arange(tmax, dtype=np.float32)[:, None] * t_w + t_b
    ).astype(np.float32)
    # Pad half of every example row: zero embed + cos(1e6*w + b) time features.
    pad_vec = np.cos(
        np.float32(PAD_TIME) * t_w.astype(np.float32) + t_b.astype(np.float32)
    ).astype(np.float32)
    pad_host = np.zeros((P, SEGS_PER_EX, H + T), np.float32)
    pad_host[:, :, H:] = pad_vec
    pad_host = np.ascontiguousarray(pad_host)

    in_maps = []
    for c in range(N_CORES):
        e0 = c * EX_PER_CORE
        fs = flat_s[
            e0 * SEGS_PER_EX * NODES_PER_SEG : (e0 + EX_PER_CORE)
            * SEGS_PER_EX
            * NODES_PER_SEG
        ].reshape(EX_PER_CORE, SEGS_PER_EX * NODES_PER_SEG)
        idx_host = np.zeros((NBLK * P, SEGS_PER_EX * NODES_PER_SEG), np.int32)
        idx_host[:EX_PER_CORE] = fs
        tvals = time_vals[
            e0 * SEGS_PER_EX : (e0 + EX_PER_CORE) * SEGS_PER_EX
        ].reshape(EX_PER_CORE, SEGS_PER_EX)
        tf_host = np.zeros((NBLK * P, SEGS_PER_EX, T), np.float32)
        tf_host[:EX_PER_CORE] = lut[tvals]
        in_maps.append(
            {
                "table": table_bf16,
                "idx": idx_host.reshape(NBLK, P, SEGS_PER_EX * NODES_PER_SEG),
                "tf": tf_host.reshape(NBLK, P, SEGS_PER_EX, T),
                "padf": pad_host,
            }
        )

    trace = os.environ.get("BASSKERNEL_TRACE", "0") == "1"
    kw = {}
    if trace:
        kw = dict(trace=True, tmpdir=os.environ.get("BASSKERNEL_TRACEDIR") or None)
    res = run_bass_kernel_spmd(nc, in_maps, core_ids=list(range(N_CORES)), **kw)
    if trace:
        _CACHE["last_results"] = res
        print(f"[kernel] exec_time_ns={res.exec_time_ns}", file=sys.stderr)

    shards = [res.results[c]["out"] for c in range(N_CORES)]
    return np.concatenate(shards, axis=0)


# revision 4
# speedup vs baseline: 1.0026x; 1.0026x over previous
"""Trainium2 Bass kernel for nn_MeanAggregator (segment mean + time features).

Computation (see reference):
  out[e, p, 0:256]   = mean of 10 gathered ent_embeds rows of segment 5e+p   (p < 5)
  out[e, p, 256:288] = cos(t * t_w + t_b), t = time_vals[5e+p]               (p < 5)
  out[e, p, 0:256]   = 0                                                      (p >= 5)
  out[e, p, 256:288] = cos(1e6 * t_w + t_b)                                   (p >= 5)

Sharding: data-parallel over examples; core c owns examples [2500c, 2500(c+1)).

Device work per core: 125k-row gather via indirect DMA (one row per
partition, example-native order so no reorder is needed), spread across the
4 SWDGE queues so descriptor generation on the Q7 pairs overlaps; the table
is pre-converted to bf16 on the host to halve gather bytes; a pairwise add
tree (bf16 -> f32) forms the 10-row segment sums; ScalarE scales into the
output tile; time features come from a host-side 300-entry cos LUT.
"""

import math
import os
import sys

import numpy as np

sys.path.insert(0, "/opt/trn_rl_repo")

from contextlib import ExitStack

import ml_dtypes

import concourse.bass as bass
import concourse.tile as tile
from concourse import bacc, mybir
from concourse._compat import with_exitstack
from concourse.bass_utils import run_bass_kernel_spmd

# Problem constants (hardcoded; kernel.py must be self-contained).
N_CORES = 8
NUM_ENTITIES = 200000
H = 256
T = 32
SEQ_LEN = 10
N_EXAMPLES = 20000
SEGS_PER_EX = 5
NODES_PER_SEG = 10
EX_PER_CORE = N_EXAMPLES // N_CORES  # 2500
P = 128
NBLK = (EX_PER_CORE + P - 1) // P  # 20
PAD_TIME = 1000000.0

N_QUEUES = 4
_QNAMES = ["qPoolDynamic", "qPoolDynamic1", "qPoolDynamic2", "qPoolDynamic3"]

_CACHE = {}


@with_exitstack
def _emit(ctx: ExitStack, tc, table, idx, tf, padfull, out):
    nc = tc.nc
    f32 = mybir.dt.float32
    bf16 = mybir.dt.bfloat16

    const_pool = ctx.enter_context(tc.tile_pool(name="const", bufs=1))
    g_pool = ctx.enter_context(tc.tile_pool(name="g", bufs=3))
    io_pool = ctx.enter_context(tc.tile_pool(name="io", bufs=3))
    out_pool = ctx.enter_context(tc.tile_pool(name="outp", bufs=3))
    acc_pool = ctx.enter_context(tc.tile_pool(name="acc", bufs=3))

    pad_t = const_pool.tile([P, SEGS_PER_EX, H + T], f32)
    nc.sync.dma_start(out=pad_t[:], in_=padfull)

    qi = 0
    for b in range(NBLK):
        npar = min(P, EX_PER_CORE - b * P)
        idx_t = io_pool.tile([P, SEGS_PER_EX * NODES_PER_SEG], mybir.dt.int32)
        nc.sync.dma_start(out=idx_t[:npar], in_=idx[b, :npar])
        out_t = out_pool.tile([P, SEGS_PER_EX, H + T], f32)
        nc.sync.dma_start(out=out_t[:npar, :, H : H + T], in_=tf[b, :npar])

        gbig = g_pool.tile([P, SEGS_PER_EX * NODES_PER_SEG, H], bf16)
        for c in range(SEGS_PER_EX * NODES_PER_SEG):
            # HW indirect DMA only honors [P, 1] offset APs (one index
            # per partition); multi-index offsets gather garbage.
            inst = nc.gpsimd.indirect_dma_start(
                out=gbig[:npar, c, :],
                out_offset=None,
                in_=table,
                in_offset=bass.IndirectOffsetOnAxis(
                    ap=idx_t[:npar, c : c + 1], axis=0
                ),
            )
            inst.ins.queue = _QNAMES[qi % N_QUEUES]
            qi += 1
        red = acc_pool.tile([P, SEGS_PER_EX, H], f32)
        nc.vector.tensor_reduce(
            red[:npar],
            gbig[:npar].rearrange("p (s r) h -> p s h r", r=NODES_PER_SEG),
            mybir.AxisListType.X,
            mybir.AluOpType.add,
        )
        nc.scalar.mul(out_t[:npar, :, 0:H], red[:npar], 1.0 / NODES_PER_SEG)

        rows = slice(b * P, b * P + npar)
        nc.sync.dma_start(out=out[rows, 0:SEGS_PER_EX, :], in_=out_t[:npar])
        nc.sync.dma_start(out=out[rows, SEGS_PER_EX:SEQ_LEN, :], in_=pad_t[:npar])


def _build_nc():
    nc = bacc.Bacc(
        "TRN2",
        target_bir_lowering=False,
        debug=False,
        enable_asserts=False,
        num_devices=N_CORES,
        num_swdge_queues=N_QUEUES,
    )
    f32 = mybir.dt.float32
    bf16 = mybir.dt.bfloat16
    table = nc.dram_tensor("table", [NUM_ENTITIES, H], bf16, kind="ExternalInput").ap()
    idx = nc.dram_tensor(
        "idx", [NBLK, P, SEGS_PER_EX * NODES_PER_SEG], mybir.dt.int32,
        kind="ExternalInput",
    ).ap()
    tf = nc.dram_tensor(
        "tf", [NBLK, P, SEGS_PER_EX, T], f32, kind="ExternalInput"
    ).ap()
    padfull = nc.dram_tensor(
        "padf", [P, SEGS_PER_EX, H + T], f32, kind="ExternalInput"
    ).ap()
    out = nc.dram_tensor(
        "out", [EX_PER_CORE, SEQ_LEN, H + T], f32, kind="ExternalOutput"
    ).ap()
    with tile.TileContext(nc) as tc:
        _emit(tc, table, idx, tf, padfull, out)
    nc.compile()
    return nc


def kernel(
    ent_embeds, t_w, t_b, flat_s, node_seg_ids, seg_example, seg_pos, time_vals
):
    ent_embeds = np.ascontiguousarray(ent_embeds, dtype=np.float32)
    t_w = np.asarray(t_w, dtype=np.float32)
    t_b = np.asarray(t_b, dtype=np.float32)
    flat_s = np.asarray(flat_s, dtype=np.int32)
    time_vals = np.asarray(time_vals, dtype=np.int32)

    if "nc" not in _CACHE:
        _CACHE["nc"] = _build_nc()
    nc = _CACHE["nc"]

    table_bf16 = np.ascontiguousarray(ent_embeds.astype(ml_dtypes.bfloat16))

    # Host-side prep. Time features take only 300 distinct integer t values:
    # precompute the 300x32 cos LUT (like an activation table) and expand.
    tmax = int(time_vals.max()) + 1
    lut = np.cos(
        np.arange(tmax, dtype=np.float32)[:, None] * t_w + t_b
    ).astype(np.float32)
    # Pad half of every example row: zero embed + cos(1e6*w + b) time features.
    pad_vec = np.cos(
        np.float32(PAD_TIME) * t_w.astype(np.float32) + t_b.astype(np.float32)
    ).astype(np.float32)
    pad_host = np.zeros((P, SEGS_PER_EX, H + T), np.float32)
    pad_host[:, :, H:] = pad_vec
    pad_host = np.ascontiguousarray(pad_host)

    in_maps = []
    for c in range(N_CORES):
        e0 = c * EX_PER_CORE
        fs = flat_s[
            e0 * SEGS_PER_EX * NODES_PER_SEG : (e0 + EX_PER_CORE)
            * SEGS_PER_EX
            * NODES_PER_SEG
        ].reshape(EX_PER_CORE, SEGS_PER_EX * NODES_PER_SEG)
        idx_host = np.zeros((NBLK * P, SEGS_PER_EX * NODES_PER_SEG), np.int32)
        idx_host[:EX_PER_CORE] = fs
        tvals = time_vals[
            e0 * SEGS_PER_EX : (e0 + EX_PER_CORE) * SEGS_PER_EX
        ].reshape(EX_PER_CORE, SEGS_PER_EX)
        tf_host = np.zeros((NBLK * P, SEGS_PER_EX, T), np.float32)
        tf_host[:EX_PER_CORE] = lut[tvals]
        in_maps.append(
            {
                "table": table_bf16,
                "idx": idx_host.reshape(NBLK, P, SEGS_PER_EX * NODES_PER_SEG),
                "tf": tf_host.reshape(NBLK, P, SEGS_PER_EX, T),
                "padf": pad_host,
            }
        )

    trace = os.environ.get("BASSKERNEL_TRACE", "0") == "1"
    kw = {}
    if trace:
        kw = dict(trace=True, tmpdir=os.environ.get("BASSKERNEL_TRACEDIR") or None)
    res = run_bass_kernel_spmd(nc, in_maps, core_ids=list(range(N_CORES)), **kw)
    if trace:
        _CACHE["last_results"] = res
        print(f"[kernel] exec_time_ns={res.exec_time_ns}", file=sys.stderr)

    shards = [res.results[c]["out"] for c in range(N_CORES)]
    return np.concatenate(shards, axis=0)


# revision 7
# speedup vs baseline: 1.3718x; 1.3683x over previous
"""Trainium2 Bass kernel for nn_MeanAggregator (segment mean + time features).

v6: batched dma_gather two-phase design.

  phase 1: bf16 table rows gathered via dma_gather in chunk-bucketed order
           (int16 index reach = 32768-row chunks), <=1024 idx per gather,
           spread across the 4 SWDGE queues -> T1 [128, B1, 256] per piece.
  phase 2: SBUF-source transposed dma_gather (<=512 tokens per gather, the
           xbar packet limit) reorders rows seg-major -> T2 [128, 2, 512].
  reduce:  DVE sums groups of 10 -> red [128, 2, 512] f32 per 500 segs.
  PE:      transpose per 125-seg group back to [seg, 256] in PSUM.
  scalar:  x0.1 into staging [128, 4, 288]; time features DMA'd into cols
           256:288; strided writes to out; pad half written per 128 examples.

All shapes are static and identical across cores: per-(piece,chunk) padded
run lengths are maxed over the 8 cores; pads gather row 0 of the chunk and
are never referenced by phase-2 tokens.
"""

import os
import sys

import numpy as np

sys.path.insert(0, "/opt/trn_rl_repo")

from contextlib import ExitStack

import ml_dtypes

import concourse.bass as bass
import concourse.tile as tile
from concourse import bacc, mybir
from concourse._compat import with_exitstack
from concourse.bass_utils import run_bass_kernel_spmd

N_CORES = 8
NUM_ENTITIES = 200000
H = 256
T = 32
SEQ_LEN = 10
N_EXAMPLES = 20000
SEGS_PER_EX = 5
NODES_PER_SEG = 10
EX_PER_CORE = N_EXAMPLES // N_CORES          # 2500
SEG_PER_CORE = EX_PER_CORE * SEGS_PER_EX     # 12500
ROWS_PER_CORE = SEG_PER_CORE * NODES_PER_SEG # 125000
P = 128
PAD_TIME = 1000000.0

CHUNK = 32768
N_CHUNKS = (NUM_ENTITIES + CHUNK - 1) // CHUNK  # 7

SEGS_PER_PIECE = 1000
SEGS_PER_BATCH = 500                         # output batch: 100 examples
SEGS_PER_SUB = 50                            # phase-2 gather: 500+12 tokens
TOK_PAD = 512
SUBS_PER_BATCH = SEGS_PER_BATCH // SEGS_PER_SUB   # 10
BATCHES_PER_CORE = SEG_PER_CORE // SEGS_PER_BATCH  # 25
N_QUEUES = 4
MAX_G1 = 1024                                # non-transpose gather idx cap

PIECES = []
_s = 0
while _s < SEG_PER_CORE:
    n = min(SEGS_PER_PIECE, SEG_PER_CORE - _s)
    PIECES.append((_s, n))
    _s += n

_CACHE = {}


def _host_prep(flat_s):
    rows_all = np.asarray(flat_s, dtype=np.int64).reshape(N_CORES, ROWS_PER_CORE)
    n_runs = np.zeros((N_CORES, len(PIECES), N_CHUNKS), np.int64)
    per_core = []
    for c in range(N_CORES):
        per_piece = []
        for pi, (s0, nseg) in enumerate(PIECES):
            rows = rows_all[c, s0 * NODES_PER_SEG : (s0 + nseg) * NODES_PER_SEG]
            chunk_of = rows >> 15
            local = (rows & (CHUNK - 1)).astype(np.int16)
            order = np.argsort(chunk_of, kind="stable")
            n_runs[c, pi] = np.bincount(chunk_of, minlength=N_CHUNKS)
            per_piece.append((local, chunk_of, order))
        per_core.append(per_piece)

    # padded block count per (piece, chunk), maxed over cores
    B = np.maximum(1, (n_runs.max(axis=0) + P - 1) // P)  # [npieces, N_CHUNKS]

    idx1_list, idx2_list = [], []
    for c in range(N_CORES):
        idx1_pieces, idx2_pieces = [], []
        for pi, (s0, nseg) in enumerate(PIECES):
            local, chunk_of, order = per_core[c][pi]
            nrow = nseg * NODES_PER_SEG
            run_off = np.concatenate([[0], np.cumsum(B[pi] * P)[:-1]])
            idx1 = np.zeros(int(B[pi].sum() * P), np.int16)
            pos = np.empty(nrow, np.int64)
            sorted_local = local[order]
            ofs = 0
            for ch in range(N_CHUNKS):
                n = int(n_runs[c, pi, ch])
                idx1[run_off[ch] : run_off[ch] + n] = sorted_local[ofs : ofs + n]
                pos[order[ofs : ofs + n]] = run_off[ch] + np.arange(n)
                ofs += n
            idx1_pieces.append(idx1)
            nsub = nseg * NODES_PER_SEG // (SEGS_PER_SUB * NODES_PER_SEG)
            toks = np.zeros((nsub, TOK_PAD), np.int16)
            pos16 = pos.astype(np.int16)
            w = SEGS_PER_SUB * NODES_PER_SEG  # 500
            for si in range(nsub):
                toks[si, :w] = pos16[si * w : (si + 1) * w]
            idx2_pieces.append(toks)
        idx1_list.append(idx1_pieces)
        idx2_list.append(idx2_pieces)
    return B, idx1_list, idx2_list


def _wrap16(idx):
    J = idx.shape[0]
    assert J % 16 == 0
    a = idx.reshape(J // 16, 16).T
    return np.tile(a, (8, 1))


@with_exitstack
def _emit(ctx: ExitStack, tc, B, tbl, idx1, idx2, tf, ident, padfull, out):
    nc = tc.nc
    f32 = mybir.dt.float32
    bf16 = mybir.dt.bfloat16
    i16 = mybir.dt.int16

    B1_all = [int(B[pi].sum()) for pi in range(len(PIECES))]
    B1_max = max(B1_all)

    cpool = ctx.enter_context(tc.tile_pool(name="const", bufs=1))
    t1pool = ctx.enter_context(tc.tile_pool(name="t1", bufs=2))
    t2pool = ctx.enter_context(tc.tile_pool(name="t2", bufs=4))
    i1pool = ctx.enter_context(tc.tile_pool(name="i1", bufs=2))
    i2pool = ctx.enter_context(tc.tile_pool(name="i2", bufs=2))
    rpool = ctx.enter_context(tc.tile_pool(name="red", bufs=2))
    spool = ctx.enter_context(tc.tile_pool(name="stg", bufs=3))
    ppool = ctx.enter_context(tc.psum_pool(name="ps", bufs=4))

    ident_t = cpool.tile([P, P], f32)
    nc.sync.dma_start(out=ident_t[:], in_=ident)
    pad_t = cpool.tile([P, SEGS_PER_EX, H + T], f32)
    nc.sync.dma_start(out=pad_t[:], in_=padfull)

    e = 0
    while e < EX_PER_CORE:
        npar = min(P, EX_PER_CORE - e)
        nc.sync.dma_start(
            out=out[e : e + npar, SEGS_PER_EX:SEQ_LEN, :], in_=pad_t[:npar]
        )
        e += npar

    # out viewed per 500-seg batch: [batch, ex25, pos, grp, col]
    out5 = out.rearrange("(s g e) p h -> s e p g h", g=4, e=25)

    qi = 0
    batch_global = 0
    for pi, (s0, nseg) in enumerate(PIECES):
        Bp = [int(x) for x in B[pi]]
        B1 = B1_all[pi]
        nbatch = nseg // SEGS_PER_BATCH
        nsub = nseg // SEGS_PER_SUB

        i1_t = i1pool.tile([P, B1_max * P // 16], i16)
        nc.sync.dma_start(out=i1_t[:, : B1 * P // 16], in_=idx1[pi])
        i2_t = i2pool.tile([P, (SEGS_PER_PIECE // SEGS_PER_SUB) * TOK_PAD // 16], i16)
        nc.sync.dma_start(out=i2_t[:, : nsub * TOK_PAD // 16], in_=idx2[pi])

        t1 = t1pool.tile([P, B1_max, H], bf16)
        off = 0
        for ch in range(N_CHUNKS):
            n = Bp[ch] * P
            base = ch * CHUNK
            nrows = min(CHUNK, NUM_ENTITIES - base)
            o = 0
            while o < n:
                m = min(MAX_G1, n - o)
                nc.gpsimd.dma_gather(
                    t1[:, (off + o) // P : (off + o + m) // P, :],
                    tbl[base : base + nrows],
                    i1_t[:, (off + o) // 16 : (off + o + m) // 16],
                    m,
                    m,
                    H,
                    queue_num=qi % N_QUEUES,
                )
                qi += 1
                o += m
            off += n

        t1flat = t1[:, :B1, :].rearrange("p a b -> p (a b)")
        for bi in range(nbatch):
            red = rpool.tile([P, 2, SEGS_PER_BATCH + 12], f32)
            for sj in range(SUBS_PER_BATCH):
                si = bi * SUBS_PER_BATCH + sj
                t2 = t2pool.tile([P, 2, TOK_PAD], bf16)
                nc.gpsimd.dma_gather(
                    t2[:],
                    t1flat,
                    i2_t[:, si * TOK_PAD // 16 : (si + 1) * TOK_PAD // 16],
                    TOK_PAD,
                    TOK_PAD,
                    H,
                    transpose=True,
                    sbuf_tokens_per_rank=P,
                    sbuf_free_dim_per_rank=H * 2,
                    queue_num=qi % N_QUEUES,
                )
                qi += 1
                nc.vector.tensor_reduce(
                    red[:, :, sj * SEGS_PER_SUB : (sj + 1) * SEGS_PER_SUB],
                    t2[:, :, 0 : SEGS_PER_SUB * NODES_PER_SEG].rearrange(
                        "p a (s r) -> p a s r", r=NODES_PER_SEG
                    ),
                    mybir.AxisListType.X,
                    mybir.AluOpType.add,
                )
            stage = spool.tile([P, 4, H + T], f32)
            nc.sync.dma_start(out=stage[:125, :, H : H + T], in_=tf[batch_global])
            for g in range(4):
                psum = ppool.tile([P, H], f32)
                for hh in range(2):
                    nc.tensor.transpose(
                        psum[0:125, hh * P : (hh + 1) * P],
                        red[:, hh, g * 125 : (g + 1) * 125],
                        ident_t[:],
                    )
                nc.scalar.mul(
                    stage[0:125, g, 0:H], psum[0:125, :], 1.0 / NODES_PER_SEG
                )
            for g in range(4):
                nc.sync.dma_start(
                    out=out5[batch_global, :, 0:SEGS_PER_EX, g, :],
                    in_=stage[0:125, g, :],
                )
            batch_global += 1


def _build_nc(B):
    nc = bacc.Bacc(
        "TRN2",
        target_bir_lowering=False,
        debug=False,
        enable_asserts=False,
        num_devices=N_CORES,
        num_swdge_queues=N_QUEUES,
    )
    f32 = mybir.dt.float32
    bf16 = mybir.dt.bfloat16
    i16 = mybir.dt.int16

    tbl = nc.dram_tensor("tbl", [NUM_ENTITIES, H], bf16, kind="ExternalInput").ap()
    idx1 = [
        nc.dram_tensor(
            f"idx1_{pi}", [P, int(B[pi].sum()) * P // 16], i16, kind="ExternalInput"
        ).ap()
        for pi in range(len(PIECES))
    ]
    idx2 = [
        nc.dram_tensor(
            f"idx2_{pi}",
            [P, (PIECES[pi][1] // SEGS_PER_SUB) * TOK_PAD // 16],
            i16,
            kind="ExternalInput",
        ).ap()
        for pi in range(len(PIECES))
    ]
    tf = nc.dram_tensor(
        "tf", [BATCHES_PER_CORE, 125, 4, T], f32, kind="ExternalInput"
    ).ap()
    ident = nc.dram_tensor("ident", [P, P], f32, kind="ExternalInput").ap()
    padfull = nc.dram_tensor(
        "padf", [P, SEGS_PER_EX, H + T], f32, kind="ExternalInput"
    ).ap()
    out = nc.dram_tensor(
        "out", [EX_PER_CORE, SEQ_LEN, H + T], f32, kind="ExternalOutput"
    ).ap()
    with tile.TileContext(nc) as tc:
        _emit(tc, B, tbl, idx1, idx2, tf, ident, padfull, out)
    nc.compile()
    return nc


def kernel(
    ent_embeds, t_w, t_b, flat_s, node_seg_ids, seg_example, seg_pos, time_vals
):
    ent_embeds = np.ascontiguousarray(ent_embeds, dtype=np.float32)
    t_w = np.asarray(t_w, dtype=np.float32)
    t_b = np.asarray(t_b, dtype=np.float32)
    flat_s = np.asarray(flat_s, dtype=np.int32)
    time_vals = np.asarray(time_vals, dtype=np.int32)

    B, idx1_list, idx2_list = _host_prep(flat_s)
    key = B.tobytes()
    if _CACHE.get("key") != key:
        _CACHE["nc"] = _build_nc(B)
        _CACHE["key"] = key
    nc = _CACHE["nc"]

    tbl_bf16 = np.ascontiguousarray(ent_embeds.astype(ml_dtypes.bfloat16))

    tmax = int(time_vals.max()) + 1
    lut = np.cos(np.arange(tmax, dtype=np.float32)[:, None] * t_w + t_b).astype(
        np.float32
    )
    pad_vec = np.cos(np.float32(PAD_TIME) * t_w + t_b).astype(np.float32)
    pad_host = np.zeros((P, SEGS_PER_EX, H + T), np.float32)
    pad_host[:, :, H:] = pad_vec
    pad_host = np.ascontiguousarray(pad_host)
    ident = np.eye(P, dtype=np.float32)

    in_maps = []
    for c in range(N_CORES):
        tvals = time_vals[c * SEG_PER_CORE : (c + 1) * SEG_PER_CORE]
        tf_seg = lut[tvals]  # [12500, 32]
        # [batch, q(125), grp, T]: seg = 500*batch + 125*grp + q
        tf_host = np.ascontiguousarray(
            tf_seg.reshape(BATCHES_PER_CORE, 4, 125, T).transpose(0, 2, 1, 3)
        )
        im = {
            "tbl": tbl_bf16,
            "tf": tf_host,
            "ident": ident,
            "padf": pad_host,
        }
        for pi in range(len(PIECES)):
            im[f"idx1_{pi}"] = np.ascontiguousarray(_wrap16(idx1_list[c][pi]))
            im[f"idx2_{pi}"] = np.ascontiguousarray(
                _wrap16(idx2_list[c][pi].reshape(-1))
            )
        in_maps.append(im)

    trace = os.environ.get("BASSKERNEL_TRACE", "0") == "1"
    kw = {}
    if trace:
        kw = dict(trace=True, tmpdir=os.environ.get("BASSKERNEL_TRACEDIR") or None)
    res = run_bass_kernel_spmd(nc, in_maps, core_ids=list(range(N_CORES)), **kw)
    if trace:
        _CACHE["last_results"] = res
        print(f"[kernel] exec_time_ns={res.exec_time_ns}", file=sys.stderr)

    shards = [res.results[c]["out"] for c in range(N_CORES)]
    return np.concatenate(shards, axis=0)
